# revision 6
# baseline (speedup 1.0000x reference)
"""AdMSoftmax loss on 8 TRN2 NeuronCores.

Strategy (vocab/tensor parallel, per sharding hint):
  - Shard the class dim C=100000 into 8 shards of 12500 (padded to 12800).
  - Host-side prep (layout only): transpose each W shard to (E, Cs) so the
    device DMAs W^T tiles [128e x c] with perfectly contiguous rows; also
    ship x^T for the matmul stationary operand.
  - Each core computes psum[n, c] = sum_e x[n,e] * W[c,e] with f32r matmuls
    (full-rate fp32 path), then ScalarE exp with per-partition scale
    S/||x_n|| fused with the free-axis row-sum (accum_out). Output: per-core
    partial sums of exp(S * wf) over its class shard, shape [128, 8]
    (partition p, row-chunk j -> row n = j*128 + p).
  - Host combines the 8 partials (the all-reduce of the denominator), adds
    the exact label term, and finishes the scalar loss.
"""

import numpy as np

N, E, C = 1024, 512, 100000
S, M = 30.0, 0.4
NCORES = 8
CS = C // NCORES            # 12500 classes per core
CPAD = 12800                # padded to a multiple of 512
PAD_PER_CORE = CPAD - CS    # each pad col contributes exp(0) = 1

GROUP_W = 2048              # psum group width (4 banks)

_nc_cache = None


def _split_bir_waits(bir_json):
    """The walrus build in this image lowers at most ONE sync-wait per
    instruction (TPB_EVENTS has a single wait slot); Tile emits tail Drains
    with several. Split extra waits into single-wait EventSemaphore preludes
    on the same engine (sequential waits == AND of waits)."""
    import orjson
    j = orjson.loads(bir_json)
    changed = False
    for fn in j.get("functions", []):
        for bb in fn.get("blocks", []):
            out = []
            for inst in bb.get("instructions", []):
                si = inst.get("sync_info") or {}
                waits = si.get("on_wait") or []
                if len(waits) > 1:
                    changed = True
                    for k, w in enumerate(waits[:-1]):
                        out.append({
                            "debug": inst.get("debug", 0),
                            "engine": inst["engine"],
                            "ins": [], "outs": [],
                            "name": f'{inst["name"]}_wsplit{k}',
                            "opcode": "EventSemaphore",
                            "sync_info": {"on_update": [], "on_wait": [w]},
                        })
                    si["on_wait"] = [waits[-1]]
                    inst["sync_info"] = si
                out.append(inst)
            bb["instructions"] = out
    return orjson.dumps(j) if changed else bir_json


def _install_compile_patch():
    from concourse import bass2jax
    if getattr(bass2jax, "_wait_split_patched", False):
        return
    orig = bass2jax.compile_bir_kernel

    def patched(bir_json, tmpdir, neff_name="file.neff"):
        return orig(_split_bir_waits(bir_json), tmpdir, neff_name)

    bass2jax.compile_bir_kernel = patched
    bass2jax._wait_split_patched = True


def _class_groups():
    groups = []
    c0 = 0
    while c0 < CPAD:
        w = min(GROUP_W, CPAD - c0)
        groups.append((c0, w))
        c0 += w
    return groups


def _build_nc():
    from concourse import bass, mybir, tile

    f32 = mybir.dt.float32
    f32r = mybir.dt.float32r
    AF = mybir.ActivationFunctionType
    ALU = mybir.AluOpType
    AX = mybir.AxisListType

    nc = bass.Bass(target_bir_lowering=False)
    x_ext = nc.declare_dram_parameter("x", [N, E], f32, isOutput=False)
    xT_ext = nc.declare_dram_parameter("xT", [E, N], f32r, isOutput=False)
    wT_ext = nc.declare_dram_parameter("wT", [E, CPAD], f32r, isOutput=False)
    out_ext = nc.declare_dram_parameter("out", [128, 8], f32, isOutput=True)

    groups = _class_groups()
    NG = len(groups)

    with tile.TileContext(nc) as tc:
        with tc.tile_pool(name="const", bufs=1) as cpool, \
             tc.tile_pool(name="wt", bufs=8) as wpool, \
             tc.tile_pool(name="ps", bufs=2, space="PSUM") as ppool, \
             tc.tile_pool(name="scr", bufs=2) as spool:

            # x rows (for norms): chunk j occupies cols [j*512, (j+1)*512)
            x_sb = cpool.tile([128, 8 * E], f32)
            for j in range(8):
                nc.sync.dma_start(x_sb[:, j * E:(j + 1) * E],
                                  x_ext[j * 128:(j + 1) * 128, :])
            # x^T (stationary): e-chunk e occupies cols [e*1024, (e+1)*1024)
            xT_sb = cpool.tile([128, 4 * N], f32r)
            for e in range(4):
                nc.sync.dma_start(xT_sb[:, e * N:(e + 1) * N],
                                  xT_ext[e * 128:(e + 1) * 128, :])

            # scale[p, j] = S / ||x_{j*128+p}||  via  S * exp(-0.5 * ln(sumsq))
            sq_scr = cpool.tile([128, E], f32)
            sumsq = cpool.tile([128, 8], f32)
            for j in range(8):
                nc.scalar.activation(
                    sq_scr[:], x_sb[:, j * E:(j + 1) * E], AF.Square,
                    accum_out=sumsq[:, j:j + 1])
            lns = cpool.tile([128, 8], f32)
            nc.scalar.activation(lns[:], sumsq[:], AF.Ln)
            rinv = cpool.tile([128, 8], f32)
            nc.scalar.activation(rinv[:], lns[:], AF.Exp, scale=-0.5)
            scale_sb = cpool.tile([128, 8], f32)
            nc.vector.tensor_scalar_mul(scale_sb[:], rinv[:], S)

            # per-(row-chunk, group) partial sums
            sums = cpool.tile([128, 8 * NG], f32)

            for gi, (c0, wdt) in enumerate(groups):
                wts = []
                for e in range(4):
                    wt = wpool.tile([128, GROUP_W], f32r, tag="wt")
                    nc.sync.dma_start(wt[:, :wdt],
                                      wT_ext[e * 128:(e + 1) * 128, c0:c0 + wdt])
                    wts.append(wt)
                for n in range(8):
                    ps = ppool.tile([128, GROUP_W], f32)
                    for e in range(4):
                        lhs = xT_sb[:, e * N + n * 128:
                                    e * N + (n + 1) * 128]
                        for b in range(wdt // 512):
                            nc.tensor.matmul(
                                ps[:, b * 512:(b + 1) * 512],
                                lhs,
                                wts[e][:, b * 512:(b + 1) * 512],
                                start=(e == 0), stop=(e == 3))
                    escr = spool.tile([128, GROUP_W], f32, tag="escr")
                    nc.scalar.activation(
                        escr[:, :wdt], ps[:, :wdt], AF.Exp,
                        scale=scale_sb[:, n:n + 1],
                        accum_out=sums[:, n * NG + gi:n * NG + gi + 1])

            partial = cpool.tile([128, 8], f32)
            for n in range(8):
                nc.vector.tensor_reduce(
                    partial[:, n:n + 1], sums[:, n * NG:(n + 1) * NG],
                    axis=AX.X, op=ALU.add)
            nc.sync.dma_start(out_ext[:, :], partial[:, :])

    return nc


TRACE = False
TRACE_KW = {}
LAST_RESULT = None


def kernel(x, labels, W):
    global _nc_cache, LAST_RESULT
    x = np.ascontiguousarray(np.asarray(x, dtype=np.float32))
    W = np.ascontiguousarray(np.asarray(W, dtype=np.float32))
    labels_i = np.asarray(labels).astype(np.int64)

    _install_compile_patch()
    if _nc_cache is None:
        _nc_cache = _build_nc()
    nc = _nc_cache

    xT = np.ascontiguousarray(x.T)
    in_maps = []
    for i in range(NCORES):
        wiT = np.zeros((E, CPAD), dtype=np.float32)
        wiT[:, :CS] = W[i * CS:(i + 1) * CS].T
        in_maps.append({"x": x, "xT": xT, "wT": wiT})

    from concourse.bass_utils import run_bass_kernel_spmd
    res = run_bass_kernel_spmd(nc, in_maps, core_ids=list(range(NCORES)),
                               trace=TRACE, **TRACE_KW)
    LAST_RESULT = res

    total = np.zeros(N, dtype=np.float64)
    for i in range(NCORES):
        o = np.asarray(res.results[i]["out"], dtype=np.float64)  # [128, 8]
        total += o.T.reshape(N)
    sum_all = total - NCORES * PAD_PER_CORE

    # Exact label term + final scalar combine (the gather/unshard step).
    xn = x.astype(np.float64)
    xn /= np.linalg.norm(xn, axis=1, keepdims=True)
    wf_y = np.sum(xn * W[labels_i].astype(np.float64), axis=1)
    numerator = S * (wf_y - M)
    denominator = np.exp(numerator) + sum_all - np.exp(S * wf_y)
    L = numerator - np.log(denominator)
    return np.float32(-np.mean(L))


# revision 9
# speedup vs baseline: 1.2552x; 1.2552x over previous
"""AdMSoftmax loss on 8 TRN2 NeuronCores.

Strategy (vocab/tensor parallel, per sharding hint):
  - Shard the class dim C=100000 into 8 shards of 12500 (padded to 12800).
  - Host-side prep (layout only): transpose each W shard to (E, Cs) so the
    device DMAs W^T tiles [128e x c] with perfectly contiguous rows; also
    ship x^T for the matmul stationary operand.
  - Each core computes psum[n, c] = sum_e x[n,e] * W[c,e] with f32r matmuls
    (full-rate fp32 path), then ScalarE exp with per-partition scale
    S/||x_n|| fused with the free-axis row-sum (accum_out). Output: per-core
    partial sums of exp(S * wf) over its class shard, shape [128, 8]
    (partition p, row-chunk j -> row n = j*128 + p).
  - Host combines the 8 partials (the all-reduce of the denominator), adds
    the exact label term, and finishes the scalar loss.
"""

import numpy as np

N, E, C = 1024, 512, 100000
S, M = 30.0, 0.4
NCORES = 8
CS = C // NCORES            # 12500 classes per core
CPAD = 12800                # padded to a multiple of 512
PAD_PER_CORE = CPAD - CS    # each pad col contributes exp(0) = 1

GROUP_W = 2048              # psum group width (4 banks)

_nc_cache = None


def _split_bir_waits(bir_json):
    """The walrus build in this image lowers at most ONE sync-wait per
    instruction (TPB_EVENTS has a single wait slot); Tile emits tail Drains
    with several. Split extra waits into single-wait EventSemaphore preludes
    on the same engine (sequential waits == AND of waits)."""
    import orjson
    j = orjson.loads(bir_json)
    changed = False
    for fn in j.get("functions", []):
        for bb in fn.get("blocks", []):
            out = []
            for inst in bb.get("instructions", []):
                si = inst.get("sync_info") or {}
                waits = si.get("on_wait") or []
                if len(waits) > 1:
                    changed = True
                    for k, w in enumerate(waits[:-1]):
                        out.append({
                            "debug": inst.get("debug", 0),
                            "engine": inst["engine"],
                            "ins": [], "outs": [],
                            "name": f'{inst["name"]}_wsplit{k}',
                            "opcode": "EventSemaphore",
                            "sync_info": {"on_update": [], "on_wait": [w]},
                        })
                    si["on_wait"] = [waits[-1]]
                    inst["sync_info"] = si
                out.append(inst)
            bb["instructions"] = out
    return orjson.dumps(j) if changed else bir_json


def _install_compile_patch():
    from concourse import bass2jax
    if getattr(bass2jax, "_wait_split_patched", False):
        return
    orig = bass2jax.compile_bir_kernel

    def patched(bir_json, tmpdir, neff_name="file.neff"):
        return orig(_split_bir_waits(bir_json), tmpdir, neff_name)

    bass2jax.compile_bir_kernel = patched
    bass2jax._wait_split_patched = True


def _class_groups():
    groups = []
    c0 = 0
    while c0 < CPAD:
        w = min(GROUP_W, CPAD - c0)
        groups.append((c0, w))
        c0 += w
    return groups


VARIANT = "bf16"  # "f32r" | "mixed" | "bf16"


def _build_nc(variant=None):
    from concourse import bass, mybir, tile

    variant = VARIANT if variant is None else variant
    f32 = mybir.dt.float32
    f32r = mybir.dt.float32r
    bf16 = mybir.dt.bfloat16
    AF = mybir.ActivationFunctionType
    ALU = mybir.AluOpType
    AX = mybir.AxisListType

    stat_dt = f32r if variant == "f32r" else bf16
    mov_dt = bf16 if variant == "bf16" else f32r

    nc = bass.Bass(target_bir_lowering=False)
    x_ext = nc.declare_dram_parameter("x", [N, E], f32, isOutput=False)
    xT_ext = nc.declare_dram_parameter("xT", [E, N], f32r, isOutput=False)
    wT_ext = nc.declare_dram_parameter("wT", [E, CPAD], f32r, isOutput=False)
    out_ext = nc.declare_dram_parameter("out", [128, 8], f32, isOutput=True)

    groups = _class_groups()
    NG = len(groups)

    with tile.TileContext(nc) as tc:
        with tc.tile_pool(name="const", bufs=1) as cpool, \
             tc.tile_pool(name="wt", bufs=8) as wpool, \
             tc.tile_pool(name="wtb", bufs=8) as wbpool, \
             tc.tile_pool(name="ps", bufs=2, space="PSUM") as ppool, \
             tc.tile_pool(name="scr", bufs=2) as spool:

            # x rows (for norms): chunk j occupies cols [j*512, (j+1)*512)
            x_sb = cpool.tile([128, 8 * E], f32)
            for j in range(8):
                nc.sync.dma_start(x_sb[:, j * E:(j + 1) * E],
                                  x_ext[j * 128:(j + 1) * 128, :])
            # x^T (stationary): e-chunk e occupies cols [e*1024, (e+1)*1024)
            xT_sb = cpool.tile([128, 4 * N], f32r)
            for e in range(4):
                nc.sync.dma_start(xT_sb[:, e * N:(e + 1) * N],
                                  xT_ext[e * 128:(e + 1) * 128, :])
            if stat_dt == bf16:
                xT_use = cpool.tile([128, 4 * N], bf16)
                nc.vector.tensor_copy(xT_use[:], xT_sb[:].bitcast(f32))
            else:
                xT_use = xT_sb

            # scale[p, j] = S / ||x_{j*128+p}||  via  S * exp(-0.5 * ln(sumsq))
            sq_scr = cpool.tile([128, E], f32)
            sumsq = cpool.tile([128, 8], f32)
            for j in range(8):
                nc.scalar.activation(
                    sq_scr[:], x_sb[:, j * E:(j + 1) * E], AF.Square,
                    accum_out=sumsq[:, j:j + 1])
            lns = cpool.tile([128, 8], f32)
            nc.scalar.activation(lns[:], sumsq[:], AF.Ln)
            rinv = cpool.tile([128, 8], f32)
            nc.scalar.activation(rinv[:], lns[:], AF.Exp, scale=-0.5)
            scale_sb = cpool.tile([128, 8], f32)
            nc.vector.tensor_scalar_mul(scale_sb[:], rinv[:], S)

            # per-(row-chunk, group) partial sums
            sums = cpool.tile([128, 8 * NG], f32)

            for gi, (c0, wdt) in enumerate(groups):
                wts = []
                for e in range(4):
                    wt = wpool.tile([128, GROUP_W], f32r, tag="wt")
                    nc.sync.dma_start(wt[:, :wdt],
                                      wT_ext[e * 128:(e + 1) * 128, c0:c0 + wdt])
                    if mov_dt == bf16:
                        wtb = wbpool.tile([128, GROUP_W], bf16, tag="wtb")
                        nc.vector.tensor_copy(wtb[:, :wdt],
                                              wt[:, :wdt].bitcast(f32))
                        wts.append(wtb)
                    else:
                        wts.append(wt)
                for n in range(8):
                    ps = ppool.tile([128, GROUP_W], f32)
                    for e in range(4):
                        lhs = xT_use[:, e * N + n * 128:
                                     e * N + (n + 1) * 128]
                        for b in range(wdt // 512):
                            nc.tensor.matmul(
                                ps[:, b * 512:(b + 1) * 512],
                                lhs,
                                wts[e][:, b * 512:(b + 1) * 512],
                                start=(e == 0), stop=(e == 3))
                    escr = spool.tile([128, GROUP_W], f32, tag="escr")
                    nc.scalar.activation(
                        escr[:, :wdt], ps[:, :wdt], AF.Exp,
                        scale=scale_sb[:, n:n + 1],
                        accum_out=sums[:, n * NG + gi:n * NG + gi + 1])

            partial = cpool.tile([128, 8], f32)
            for n in range(8):
                nc.vector.tensor_reduce(
                    partial[:, n:n + 1], sums[:, n * NG:(n + 1) * NG],
                    axis=AX.X, op=ALU.add)
            nc.sync.dma_start(out_ext[:, :], partial[:, :])

    return nc


TRACE = False
TRACE_KW = {}
LAST_RESULT = None


def kernel(x, labels, W):
    global _nc_cache, LAST_RESULT
    x = np.ascontiguousarray(np.asarray(x, dtype=np.float32))
    W = np.ascontiguousarray(np.asarray(W, dtype=np.float32))
    labels_i = np.asarray(labels).astype(np.int64)

    _install_compile_patch()
    if _nc_cache is None or _nc_cache[0] != VARIANT:
        _nc_cache = (VARIANT, _build_nc(VARIANT))
    nc = _nc_cache[1]

    xT = np.ascontiguousarray(x.T)
    in_maps = []
    for i in range(NCORES):
        wiT = np.zeros((E, CPAD), dtype=np.float32)
        wiT[:, :CS] = W[i * CS:(i + 1) * CS].T
        in_maps.append({"x": x, "xT": xT, "wT": wiT})

    from concourse.bass_utils import run_bass_kernel_spmd
    res = run_bass_kernel_spmd(nc, in_maps, core_ids=list(range(NCORES)),
                               trace=TRACE, **TRACE_KW)
    LAST_RESULT = res

    total = np.zeros(N, dtype=np.float64)
    for i in range(NCORES):
        o = np.asarray(res.results[i]["out"], dtype=np.float64)  # [128, 8]
        total += o.T.reshape(N)
    sum_all = total - NCORES * PAD_PER_CORE

    # Exact label term + final scalar combine (the gather/unshard step).
    xn = x.astype(np.float64)
    xn /= np.linalg.norm(xn, axis=1, keepdims=True)
    wf_y = np.sum(xn * W[labels_i].astype(np.float64), axis=1)
    numerator = S * (wf_y - M)
    denominator = np.exp(numerator) + sum_all - np.exp(S * wf_y)
    L = numerator - np.log(denominator)
    return np.float32(-np.mean(L))


# revision 14
# speedup vs baseline: 1.4501x; 1.1553x over previous
"""AdMSoftmax loss on 8 TRN2 NeuronCores.

Strategy (vocab/tensor parallel, per sharding hint):
  - Shard the class dim C=100000 into 8 shards of 12500 (padded to 12800).
  - Host-side prep (layout only): transpose each W shard to (E, Cs) so the
    device DMAs W^T tiles [128e x c] with perfectly contiguous rows; also
    ship x^T for the matmul stationary operand.
  - Each core computes psum[n, c] = sum_e x[n,e] * W[c,e] with f32r matmuls
    (full-rate fp32 path), then ScalarE exp with per-partition scale
    S/||x_n|| fused with the free-axis row-sum (accum_out). Output: per-core
    partial sums of exp(S * wf) over its class shard, shape [128, 8]
    (partition p, row-chunk j -> row n = j*128 + p).
  - Host combines the 8 partials (the all-reduce of the denominator), adds
    the exact label term, and finishes the scalar loss.
"""

import numpy as np

N, E, C = 1024, 512, 100000
S, M = 30.0, 0.4
NCORES = 8
CS = C // NCORES            # 12500 classes per core
CPAD = 12800                # padded to a multiple of 512
PAD_PER_CORE = CPAD - CS    # each pad col contributes exp(0) = 1

GROUP_W = 2048              # psum group width (4 banks)

_nc_cache = None


def _split_bir_waits(bir_json):
    """The walrus build in this image lowers at most ONE sync-wait per
    instruction (TPB_EVENTS has a single wait slot); Tile emits tail Drains
    with several. Split extra waits into single-wait EventSemaphore preludes
    on the same engine (sequential waits == AND of waits)."""
    import orjson
    j = orjson.loads(bir_json)
    changed = False
    for fn in j.get("functions", []):
        for bb in fn.get("blocks", []):
            out = []
            for inst in bb.get("instructions", []):
                si = inst.get("sync_info") or {}
                waits = si.get("on_wait") or []
                if len(waits) > 1:
                    changed = True
                    for k, w in enumerate(waits[:-1]):
                        out.append({
                            "debug": inst.get("debug", 0),
                            "engine": inst["engine"],
                            "ins": [], "outs": [],
                            "name": f'{inst["name"]}_wsplit{k}',
                            "opcode": "EventSemaphore",
                            "sync_info": {"on_update": [], "on_wait": [w]},
                        })
                    si["on_wait"] = [waits[-1]]
                    inst["sync_info"] = si
                out.append(inst)
            bb["instructions"] = out
    return orjson.dumps(j) if changed else bir_json


def _install_compile_patch():
    from concourse import bass2jax
    if getattr(bass2jax, "_wait_split_patched", False):
        return
    orig = bass2jax.compile_bir_kernel

    def patched(bir_json, tmpdir, neff_name="file.neff"):
        return orig(_split_bir_waits(bir_json), tmpdir, neff_name)

    bass2jax.compile_bir_kernel = patched
    bass2jax._wait_split_patched = True


def _class_groups():
    # Ramp the first groups so the PE pipeline starts before the full 4MB
    # W-group DMA lands; steady-state groups are 2048 (4 PSUM banks).
    widths = [512, 512, 1024] + [GROUP_W] * 5 + [512]
    assert sum(widths) == CPAD
    groups = []
    c0 = 0
    for w in widths:
        groups.append((c0, w))
        c0 += w
    return groups


VARIANT = "fp8"  # "f32r" | "mixed" | "bf16"


def _build_nc(variant=None):
    from concourse import bass, mybir, tile

    variant = VARIANT if variant is None else variant
    f32 = mybir.dt.float32
    f32r = mybir.dt.float32r
    bf16 = mybir.dt.bfloat16
    fp8 = mybir.dt.float8e4
    AF = mybir.ActivationFunctionType
    ALU = mybir.AluOpType
    AX = mybir.AxisListType
    PM = mybir.MatmulPerfMode

    FP8_SCALE = 16.0  # pre-scale into e4m3's happy range; undone in exp scale
    stat_dt = f32r if variant == "f32r" else (fp8 if variant == "fp8" else bf16)
    mov_dt = f32r if variant == "f32r" else (fp8 if variant == "fp8" else bf16)

    nc = bass.Bass(target_bir_lowering=False)
    x_ext = nc.declare_dram_parameter("x", [N, E], f32, isOutput=False)
    xT_ext = nc.declare_dram_parameter("xT", [E, N], f32r, isOutput=False)
    wT_ext = nc.declare_dram_parameter("wT", [E, CPAD], f32r, isOutput=False)
    out_ext = nc.declare_dram_parameter("out", [128, 8], f32, isOutput=True)

    groups = _class_groups()
    NG = len(groups)

    with tile.TileContext(nc) as tc:
        with tc.tile_pool(name="const", bufs=1) as cpool, \
             tc.tile_pool(name="wt", bufs=8) as wpool, \
             tc.tile_pool(name="wtb", bufs=8) as wbpool, \
             tc.tile_pool(name="ps", bufs=2, space="PSUM") as ppool, \
             tc.tile_pool(name="scr", bufs=2) as spool:

            # x rows (for norms): chunk j occupies cols [j*512, (j+1)*512)
            x_sb = cpool.tile([128, 8 * E], f32)
            for j in range(8):
                nc.sync.dma_start(x_sb[:, j * E:(j + 1) * E],
                                  x_ext[j * 128:(j + 1) * 128, :])
            # x^T (stationary): e-chunk e occupies cols [e*1024, (e+1)*1024)
            xT_sb = cpool.tile([128, 4 * N], f32r)
            for e in range(4):
                nc.sync.dma_start(xT_sb[:, e * N:(e + 1) * N],
                                  xT_ext[e * 128:(e + 1) * 128, :])
            if stat_dt == bf16:
                xT_use = cpool.tile([128, 4 * N], bf16)
                nc.vector.tensor_copy(xT_use[:], xT_sb[:].bitcast(f32))
            elif stat_dt == fp8:
                xT_use = cpool.tile([128, 4 * N], fp8)
                nc.vector.tensor_scalar_mul(xT_use[:], xT_sb[:].bitcast(f32),
                                            FP8_SCALE)
            else:
                xT_use = xT_sb

            # scale[p, j] = S / ||x_{j*128+p}||  via  S * exp(-0.5 * ln(sumsq))
            # sum-of-squares on DVE (keep ScalarE free for the exp stream)
            sq_scr = cpool.tile([128, 8 * E], f32)
            nc.vector.tensor_tensor(sq_scr[:], x_sb[:], x_sb[:], ALU.mult)
            sumsq = cpool.tile([128, 8], f32)
            for j in range(8):
                nc.vector.tensor_reduce(
                    sumsq[:, j:j + 1], sq_scr[:, j * E:(j + 1) * E],
                    axis=AX.X, op=ALU.add)
            lns = cpool.tile([128, 8], f32)
            nc.scalar.activation(lns[:], sumsq[:], AF.Ln)
            rinv = cpool.tile([128, 8], f32)
            nc.scalar.activation(rinv[:], lns[:], AF.Exp, scale=-0.5)
            scale_sb = cpool.tile([128, 8], f32)
            exp_scale = S / (FP8_SCALE * FP8_SCALE) if variant == "fp8" else S
            nc.vector.tensor_scalar_mul(scale_sb[:], rinv[:], exp_scale)

            # per-(row-chunk, group) partial sums
            sums = cpool.tile([128, 8 * NG], f32)

            for gi, (c0, wdt) in enumerate(groups):
                nb = wdt // 512
                wts = []
                w8s = []
                for e in range(4):
                    wt = wpool.tile([128, GROUP_W], f32r, tag="wt")
                    nc.sync.dma_start(wt[:, :wdt],
                                      wT_ext[e * 128:(e + 1) * 128, c0:c0 + wdt])
                    if mov_dt == bf16:
                        wtb = wbpool.tile([128, GROUP_W], bf16, tag="wtb")
                        nc.vector.tensor_copy(wtb[:, :wdt],
                                              wt[:, :wdt].bitcast(f32))
                        wts.append(wtb)
                    elif mov_dt == fp8:
                        # pair tile P=e//2, plane j=e%2 at cols [j*GW, j*GW+wdt)
                        P, pj = e // 2, e % 2
                        if pj == 0:
                            w8 = wbpool.tile([128, 2 * GROUP_W], fp8, tag="w8")
                            w8s.append(w8)
                        nc.vector.tensor_scalar_mul(
                            w8s[P][:, pj * GROUP_W:pj * GROUP_W + wdt],
                            wt[:, :wdt].bitcast(f32), FP8_SCALE)
                        wts.append(wt)
                    else:
                        wts.append(wt)
                for n in range(8):
                    ps = ppool.tile([128, GROUP_W], f32)
                    if mov_dt == fp8:
                        for P in range(2):
                            lhs = xT_use[:, 2 * P * N:2 * (P + 1) * N] \
                                .rearrange("p (j q) -> p j q", j=2) \
                                [:, :, n * 128:(n + 1) * 128]
                            rhs_all = w8s[P][:, :] \
                                .rearrange("p (j c) -> p j c", j=2)
                            for b in range(nb):
                                nc.tensor.matmul(
                                    ps[:, b * 512:(b + 1) * 512],
                                    lhs,
                                    rhs_all[:, :, b * 512:(b + 1) * 512],
                                    perf_mode=PM.DoubleRow,
                                    start=(P == 0), stop=(P == 1))
                    else:
                        for e in range(4):
                            lhs = xT_use[:, e * N + n * 128:
                                         e * N + (n + 1) * 128]
                            for b in range(nb):
                                nc.tensor.matmul(
                                    ps[:, b * 512:(b + 1) * 512],
                                    lhs,
                                    wts[e][:, b * 512:(b + 1) * 512],
                                    start=(e == 0), stop=(e == 3))
                    escr = spool.tile([128, GROUP_W], f32, tag="escr")
                    nc.scalar.activation(
                        escr[:, :wdt], ps[:, :wdt], AF.Exp,
                        scale=scale_sb[:, n:n + 1],
                        accum_out=sums[:, n * NG + gi:n * NG + gi + 1])

            partial = cpool.tile([128, 8], f32)
            for n in range(8):
                nc.vector.tensor_reduce(
                    partial[:, n:n + 1], sums[:, n * NG:(n + 1) * NG],
                    axis=AX.X, op=ALU.add)
            nc.sync.dma_start(out_ext[:, :], partial[:, :])

    return nc


TRACE = False
TRACE_KW = {}
LAST_RESULT = None


def kernel(x, labels, W):
    global _nc_cache, LAST_RESULT
    x = np.ascontiguousarray(np.asarray(x, dtype=np.float32))
    W = np.ascontiguousarray(np.asarray(W, dtype=np.float32))
    labels_i = np.asarray(labels).astype(np.int64)

    _install_compile_patch()
    if _nc_cache is None or _nc_cache[0] != VARIANT:
        _nc_cache = (VARIANT, _build_nc(VARIANT))
    nc = _nc_cache[1]

    xT = np.ascontiguousarray(x.T)
    in_maps = []
    for i in range(NCORES):
        wiT = np.zeros((E, CPAD), dtype=np.float32)
        wiT[:, :CS] = W[i * CS:(i + 1) * CS].T
        in_maps.append({"x": x, "xT": xT, "wT": wiT})

    from concourse.bass_utils import run_bass_kernel_spmd
    res = run_bass_kernel_spmd(nc, in_maps, core_ids=list(range(NCORES)),
                               trace=TRACE, **TRACE_KW)
    LAST_RESULT = res

    total = np.zeros(N, dtype=np.float64)
    for i in range(NCORES):
        o = np.asarray(res.results[i]["out"], dtype=np.float64)  # [128, 8]
        total += o.T.reshape(N)
    sum_all = total - NCORES * PAD_PER_CORE

    # Exact label term + final scalar combine (the gather/unshard step).
    xn = x.astype(np.float64)
    xn /= np.linalg.norm(xn, axis=1, keepdims=True)
    wf_y = np.sum(xn * W[labels_i].astype(np.float64), axis=1)
    numerator = S * (wf_y - M)
    denominator = np.exp(numerator) + sum_all - np.exp(S * wf_y)
    L = numerator - np.log(denominator)
    return np.float32(-np.mean(L))


# revision 17
# speedup vs baseline: 1.5820x; 1.0910x over previous
"""AdMSoftmax loss on 8 TRN2 NeuronCores.

Strategy (vocab/tensor parallel, per sharding hint):
  - Shard the class dim C=100000 into 8 shards of 12500 (padded to 12800).
  - Host-side prep (layout only): transpose each W shard to (E, Cs) so the
    device DMAs W^T tiles [128e x c] with perfectly contiguous rows; also
    ship x^T for the matmul stationary operand.
  - Each core computes psum[n, c] = sum_e x[n,e] * W[c,e] with f32r matmuls
    (full-rate fp32 path), then ScalarE exp with per-partition scale
    S/||x_n|| fused with the free-axis row-sum (accum_out). Output: per-core
    partial sums of exp(S * wf) over its class shard, shape [128, 8]
    (partition p, row-chunk j -> row n = j*128 + p).
  - Host combines the 8 partials (the all-reduce of the denominator), adds
    the exact label term, and finishes the scalar loss.
"""

import numpy as np

N, E, C = 1024, 512, 100000
S, M = 30.0, 0.4
NCORES = 8
CS = C // NCORES            # 12500 classes per core
CPAD = 12800                # padded to a multiple of 512
PAD_PER_CORE = CPAD - CS    # each pad col contributes exp(0) = 1

GROUP_W = 2048              # psum group width (4 banks)

_nc_cache = None


def _split_bir_waits(bir_json):
    """The walrus build in this image lowers at most ONE sync-wait per
    instruction (TPB_EVENTS has a single wait slot); Tile emits tail Drains
    with several. Split extra waits into single-wait EventSemaphore preludes
    on the same engine (sequential waits == AND of waits)."""
    import orjson
    j = orjson.loads(bir_json)
    changed = False
    for fn in j.get("functions", []):
        for bb in fn.get("blocks", []):
            out = []
            for inst in bb.get("instructions", []):
                si = inst.get("sync_info") or {}
                waits = si.get("on_wait") or []
                if len(waits) > 1:
                    changed = True
                    for k, w in enumerate(waits[:-1]):
                        out.append({
                            "debug": inst.get("debug", 0),
                            "engine": inst["engine"],
                            "ins": [], "outs": [],
                            "name": f'{inst["name"]}_wsplit{k}',
                            "opcode": "EventSemaphore",
                            "sync_info": {"on_update": [], "on_wait": [w]},
                        })
                    si["on_wait"] = [waits[-1]]
                    inst["sync_info"] = si
                out.append(inst)
            bb["instructions"] = out
    return orjson.dumps(j) if changed else bir_json


def _install_compile_patch():
    from concourse import bass2jax
    if getattr(bass2jax, "_wait_split_patched", False):
        return
    orig = bass2jax.compile_bir_kernel

    def patched(bir_json, tmpdir, neff_name="file.neff"):
        return orig(_split_bir_waits(bir_json), tmpdir, neff_name)

    bass2jax.compile_bir_kernel = patched
    bass2jax._wait_split_patched = True


def _class_groups():
    # Ramp the first groups so the PE pipeline starts before the full 4MB
    # W-group DMA lands; steady-state groups are 2048 (4 PSUM banks).
    widths = [512, 512, 1024] + [GROUP_W] * 5 + [512]
    assert sum(widths) == CPAD
    groups = []
    c0 = 0
    for w in widths:
        groups.append((c0, w))
        c0 += w
    return groups


VARIANT = "fp8"  # "f32r" | "mixed" | "bf16"


def _build_nc(variant=None):
    from concourse import bass, mybir, tile

    variant = VARIANT if variant is None else variant
    f32 = mybir.dt.float32
    f32r = mybir.dt.float32r
    bf16 = mybir.dt.bfloat16
    fp8 = mybir.dt.float8e4
    AF = mybir.ActivationFunctionType
    ALU = mybir.AluOpType
    AX = mybir.AxisListType
    PM = mybir.MatmulPerfMode

    FP8_SCALE = 16.0  # pre-scale into e4m3's happy range; undone in exp scale
    stat_dt = f32r if variant == "f32r" else (fp8 if variant == "fp8" else bf16)
    mov_dt = f32r if variant == "f32r" else (fp8 if variant == "fp8" else bf16)

    nc = bass.Bass(target_bir_lowering=False)
    x_ext = nc.declare_dram_parameter("x", [N, E], f32, isOutput=False)
    xT_ext = nc.declare_dram_parameter("xT", [E, N], f32r, isOutput=False)
    wT_ext = nc.declare_dram_parameter("wT", [E, CPAD], f32r, isOutput=False)
    out_ext = nc.declare_dram_parameter("out", [128, 8], f32, isOutput=True)

    groups = _class_groups()
    NG = len(groups)

    with tile.TileContext(nc) as tc:
        with tc.tile_pool(name="const", bufs=1) as cpool, \
             tc.tile_pool(name="wt", bufs=8) as wpool, \
             tc.tile_pool(name="wtb", bufs=8) as wbpool, \
             tc.tile_pool(name="ps", bufs=2, space="PSUM") as ppool, \
             tc.tile_pool(name="scr", bufs=2) as spool:

            # x^T (stationary): e-chunk e occupies cols [e*1024, (e+1)*1024)
            # Issued on gpsimd so the W-group DMAs (sync engine) aren't
            # queued behind them (DMA issue costs ~0.7us each on its engine).
            xT_sb = cpool.tile([128, 4 * N], f32r)
            if stat_dt == bf16:
                xT_use = cpool.tile([128, 4 * N], bf16)
            elif stat_dt == fp8:
                xT_use = cpool.tile([128, 4 * N], fp8)
            else:
                xT_use = xT_sb
            for e in range(4):
                h = N // 2
                for hh in range(2):
                    nc.gpsimd.dma_start(
                        xT_sb[:, e * N + hh * h:e * N + (hh + 1) * h],
                        xT_ext[e * 128:(e + 1) * 128, hh * h:(hh + 1) * h])
                if stat_dt == bf16:
                    nc.vector.tensor_copy(
                        xT_use[:, e * N:(e + 1) * N],
                        xT_sb[:, e * N:(e + 1) * N].bitcast(f32))
                elif stat_dt == fp8:
                    nc.vector.tensor_scalar_mul(
                        xT_use[:, e * N:(e + 1) * N],
                        xT_sb[:, e * N:(e + 1) * N].bitcast(f32), FP8_SCALE)

            # x rows (for norms): chunk j occupies cols [j*512, (j+1)*512)
            # scale[p, j] = S / ||x_{j*128+p}||  via  S * exp(-0.5 * ln(sumsq))
            # sum-of-squares on DVE (keep ScalarE free for the exp stream)
            x_sb = cpool.tile([128, 8 * E], f32)
            sq_scr = cpool.tile([128, 8 * E], f32)
            sumsq = cpool.tile([128, 8], f32)
            for j in range(8):
                nc.gpsimd.dma_start(x_sb[:, j * E:(j + 1) * E],
                                    x_ext[j * 128:(j + 1) * 128, :])
                nc.vector.tensor_tensor(
                    sq_scr[:, j * E:(j + 1) * E], x_sb[:, j * E:(j + 1) * E],
                    x_sb[:, j * E:(j + 1) * E], ALU.mult)
                nc.vector.tensor_reduce(
                    sumsq[:, j:j + 1], sq_scr[:, j * E:(j + 1) * E],
                    axis=AX.X, op=ALU.add)
            lns = cpool.tile([128, 8], f32)
            nc.scalar.activation(lns[:], sumsq[:], AF.Ln)
            rinv = cpool.tile([128, 8], f32)
            nc.scalar.activation(rinv[:], lns[:], AF.Exp, scale=-0.5)
            scale_sb = cpool.tile([128, 8], f32)
            exp_scale = S / (FP8_SCALE * FP8_SCALE) if variant == "fp8" else S
            nc.vector.tensor_scalar_mul(scale_sb[:], rinv[:], exp_scale)

            # per-(row-chunk, group) partial sums
            sums = cpool.tile([128, 8 * NG], f32)

            for gi, (c0, wdt) in enumerate(groups):
                nb = wdt // 512
                wts = []
                w8s = []
                for e in range(4):
                    wt = wpool.tile([128, GROUP_W], f32r, tag="wt")
                    nc.sync.dma_start(wt[:, :wdt],
                                      wT_ext[e * 128:(e + 1) * 128, c0:c0 + wdt])
                    if mov_dt == bf16:
                        wtb = wbpool.tile([128, GROUP_W], bf16, tag="wtb")
                        nc.vector.tensor_copy(wtb[:, :wdt],
                                              wt[:, :wdt].bitcast(f32))
                        wts.append(wtb)
                    elif mov_dt == fp8:
                        # pair tile P=e//2, plane j=e%2 at cols [j*GW, j*GW+wdt)
                        P, pj = e // 2, e % 2
                        if pj == 0:
                            w8 = wbpool.tile([128, 2 * GROUP_W], fp8, tag="w8")
                            w8s.append(w8)
                        nc.vector.tensor_scalar_mul(
                            w8s[P][:, pj * GROUP_W:pj * GROUP_W + wdt],
                            wt[:, :wdt].bitcast(f32), FP8_SCALE)
                        wts.append(wt)
                    else:
                        wts.append(wt)
                for n in range(8):
                    ps = ppool.tile([128, GROUP_W], f32)
                    if mov_dt == fp8:
                        for P in range(2):
                            lhs = xT_use[:, 2 * P * N:2 * (P + 1) * N] \
                                .rearrange("p (j q) -> p j q", j=2) \
                                [:, :, n * 128:(n + 1) * 128]
                            rhs_all = w8s[P][:, :] \
                                .rearrange("p (j c) -> p j c", j=2)
                            for b in range(nb):
                                nc.tensor.matmul(
                                    ps[:, b * 512:(b + 1) * 512],
                                    lhs,
                                    rhs_all[:, :, b * 512:(b + 1) * 512],
                                    perf_mode=PM.DoubleRow,
                                    start=(P == 0), stop=(P == 1))
                    else:
                        for e in range(4):
                            lhs = xT_use[:, e * N + n * 128:
                                         e * N + (n + 1) * 128]
                            for b in range(nb):
                                nc.tensor.matmul(
                                    ps[:, b * 512:(b + 1) * 512],
                                    lhs,
                                    wts[e][:, b * 512:(b + 1) * 512],
                                    start=(e == 0), stop=(e == 3))
                    escr = spool.tile([128, GROUP_W], f32, tag="escr")
                    nc.scalar.activation(
                        escr[:, :wdt], ps[:, :wdt], AF.Exp,
                        scale=scale_sb[:, n:n + 1],
                        accum_out=sums[:, n * NG + gi:n * NG + gi + 1])

            partial = cpool.tile([128, 8], f32)
            nc.vector.tensor_reduce(
                partial[:, :],
                sums[:, :].rearrange("p (n g) -> p n g", n=8),
                axis=AX.X, op=ALU.add)
            nc.sync.dma_start(out_ext[:, :], partial[:, :])

    return nc


TRACE = False
TRACE_KW = {}
LAST_RESULT = None


def kernel(x, labels, W):
    global _nc_cache, LAST_RESULT
    x = np.ascontiguousarray(np.asarray(x, dtype=np.float32))
    W = np.ascontiguousarray(np.asarray(W, dtype=np.float32))
    labels_i = np.asarray(labels).astype(np.int64)

    _install_compile_patch()
    if _nc_cache is None or _nc_cache[0] != VARIANT:
        _nc_cache = (VARIANT, _build_nc(VARIANT))
    nc = _nc_cache[1]

    xT = np.ascontiguousarray(x.T)
    in_maps = []
    for i in range(NCORES):
        wiT = np.zeros((E, CPAD), dtype=np.float32)
        wiT[:, :CS] = W[i * CS:(i + 1) * CS].T
        in_maps.append({"x": x, "xT": xT, "wT": wiT})

    from concourse.bass_utils import run_bass_kernel_spmd
    res = run_bass_kernel_spmd(nc, in_maps, core_ids=list(range(NCORES)),
                               trace=TRACE, **TRACE_KW)
    LAST_RESULT = res

    total = np.zeros(N, dtype=np.float64)
    for i in range(NCORES):
        o = np.asarray(res.results[i]["out"], dtype=np.float64)  # [128, 8]
        total += o.T.reshape(N)
    sum_all = total - NCORES * PAD_PER_CORE

    # Exact label term + final scalar combine (the gather/unshard step).
    xn = x.astype(np.float64)
    xn /= np.linalg.norm(xn, axis=1, keepdims=True)
    wf_y = np.sum(xn * W[labels_i].astype(np.float64), axis=1)
    numerator = S * (wf_y - M)
    denominator = np.exp(numerator) + sum_all - np.exp(S * wf_y)
    L = numerator - np.log(denominator)
    return np.float32(-np.mean(L))
